# revision 17
# baseline (speedup 1.0000x reference)
"""Grouped-experts SwiGLU MoE kernel for 8 Trainium2 NeuronCores.

Problem: x[16384, 2048] routed to 64 experts (256 contiguous tokens each);
per expert e: out_e = (silu(x_e @ w1[e]) * (x_e @ w3[e])) @ w2[e].

Sharding: expert-parallel. Core c owns experts 8c..8c+7 and therefore tokens
[2048c, 2048(c+1)) — tokens are pre-permuted so no all-to-all is needed: each
core computes its own token slice fully locally.

This kernel is balanced at the DMA/PE intersection: fp16 weights/activations
halve HBM traffic vs fp32 (112 MiB/core -> ~326 us at 360 GB/s) while fp16
matmuls run at the full 1 row/cycle PE rate (98304 rows/expert ->
~328 us/core at 2.4 GHz). x is pre-transposed on the host so no PE
transposes are needed on device.

Per-core device program (8 experts):
  per expert:
    1. DMA xT[e] (host-pretransposed, [dim, tok] k-tiles), 1 MiB fp16;
       expert 0 is loaded in 4 chunks interleaved with its first w13 chunks
       to shorten pipeline fill; later experts prefetch xT during the
       previous expert's stage 1 to avoid queue head-of-line blocking.
    2. gu accumulation (w13 stationary, xT moving, fp16 matmuls):
       g^T/u^T [128 hid, 256 tok] accumulate over 16 k-tiles in 8 PSUM banks
    3. h^T = silu(g^T) * u^T  (ACT silu + DVE mul) -> fp16 SBUF
    4. out = h @ w2 via (hT stationary, w2 moving): out[tok, dim] natural
       layout, accumulated over 8 hidden k-tiles, n-major; each 512-wide
       n-chunk is DMA'd out as soon as it is copied so the tail is short.
Weights are host-repacked fp16 so every DMA moves contiguous >=512B chunks.
Accuracy: fp16 quantization of w/x/out gives ~1e-3 rel err.
"""

import numpy as np

import concourse.bacc as bacc
import concourse.mybir as mybir
from concourse.bass_utils import run_bass_kernel_spmd

N_CORES = 8
E_PER_CORE = 8          # experts per core
TOK_PER_E = 256         # tokens per expert
DIM = 2048
HID = 1024
P = 128
KT = DIM // P           # 16 k-tiles (contraction over dim)
KT2 = HID // P          # 8 k-tiles (contraction over hidden)
MT = HID // P           # 8 hidden m-tiles in stage 1
NCH = DIM // 512        # 4 output n-chunks of 512 in stage 2

F32 = mybir.dt.float32
F16 = mybir.dt.float16
SILU = mybir.ActivationFunctionType.Silu
MULT = mybir.AluOpType.mult

_program_cache = {}


def _build_program():
    """Per-core Bass program. Same program for all 8 cores (SPMD)."""
    from concourse.tile import TileContext

    nc = bacc.Bacc("TRN2", target_bir_lowering=False, debug=False)

    # xT: per (e, k-tile): [128 dim, 256 tok] fp16, host-pretransposed
    xT_d = nc.dram_tensor("xT", [E_PER_CORE * KT * P, TOK_PER_E], F16,
                          kind="ExternalInput")
    # w13: per (e, k-tile): [128, 2048] = [w1 k-block | w3 k-block], contiguous
    w13_d = nc.dram_tensor("w13", [E_PER_CORE * KT * P, 2 * HID], F16,
                           kind="ExternalInput")
    # w2p: per (e, n, k2): [128, 512] contiguous; (e, n) group = 1 MiB fp16
    w2_d = nc.dram_tensor("w2p", [E_PER_CORE * NCH * KT2 * P, 512], F16,
                          kind="ExternalInput")
    out_d = nc.dram_tensor("out", [E_PER_CORE * TOK_PER_E, DIM], F16,
                           kind="ExternalOutput")

    with TileContext(nc) as tc:
        with tc.tile_pool(name="const", bufs=1) as constp, \
             tc.tile_pool(name="xT", bufs=2) as xT_p, \
             tc.tile_pool(name="xT0", bufs=8) as xT0_p, \
             tc.tile_pool(name="w13", bufs=6) as w13_p, \
             tc.tile_pool(name="w2", bufs=4) as w2_p, \
             tc.tile_pool(name="hT", bufs=16) as hT_p, \
             tc.tile_pool(name="gs", bufs=3) as gs_p, \
             tc.tile_pool(name="osb", bufs=3) as osb_p, \
             tc.tile_pool(name="ps", bufs=8, space="PSUM") as ps_p:

            zbias = constp.tile([P, 1], F32)
            nc.vector.memset(zbias, 0.0)
            warm = constp.tile([P, P], F16)
            nc.vector.memset(warm, 0.0)

            # Warm the PE clock gate (HAM) with dummy matmuls during the
            # initial DMA fill: ~3.5us of back-to-back PE activity lifts the
            # clock from 1.2GHz to 2.4GHz before the real matmuls start.
            wps = ps_p.tile([P, 512], F32, tag="ps")
            for _ in range(30):
                nc.tensor.matmul(wps[:, 0:P], lhsT=warm[:], rhs=warm[:],
                                 start=True, stop=True)

            # xT[e] in SBUF: list of (tile, k0, klen) chunks, loaded on the
            # DVE queue (sync carries only weights; scalar only silu).
            # Expert 0 uses 8 independent 2-k-tile chunks so the first
            # matmul only waits for one small DMA; later experts load one
            # tile in 4 chunk DMAs, prefetched during the previous expert's
            # stage 2 (after its w2 loads are underway).
            def load_xT_whole(e):
                xt = xT_p.tile([P, KT, TOK_PER_E], F16, tag="xT")
                for c in range(4):
                    k0 = 4 * c
                    r0 = (e * KT + k0) * P
                    nc.gpsimd.dma_start(
                        out=xt[:, k0:k0 + 4, :],
                        in_=xT_d[r0:r0 + 4 * P, :].rearrange(
                            "(k p) t -> p k t", p=P),
                    )
                return [(xt, 0, KT)]

            def load_xT_chunked(e):
                chunks = []
                for c in range(KT // 2):
                    xt = xT0_p.tile([P, 2, TOK_PER_E], F16, tag="xT0")
                    r0 = (e * KT + 2 * c) * P
                    nc.gpsimd.dma_start(
                        out=xt[:],
                        in_=xT_d[r0:r0 + 2 * P, :].rearrange(
                            "(k p) t -> p k t", p=P),
                    )
                    chunks.append((xt, 2 * c, 2))
                return chunks

            xts = [None] * E_PER_CORE

            for e in range(E_PER_CORE):
                tok0 = e * TOK_PER_E
                xch = xts[e] if xts[e] is not None else load_xT_chunked(e)

                def xk(k):
                    for tile, k0, klen in xch:
                        if k0 <= k < k0 + klen:
                            return tile[:, k - k0, :]
                    raise AssertionError

                # ---- stage 1: g/u accumulation over dim ----
                gu = [ps_p.tile([P, 512], F32, tag="ps", name=f"gu_e{e}_m{m}")
                      for m in range(MT)]
                for kk in range(KT // 2):
                    wt = w13_p.tile([P, 2, 2 * HID], F16, tag="w13")
                    row0 = (e * KT + 2 * kk) * P
                    nc.sync.dma_start(
                        out=wt[:],
                        in_=w13_d[row0:row0 + 2 * P, :].rearrange(
                            "(h p) c -> p h c", p=P),
                    )
                    halves = [wt[:, 0, :], wt[:, 1, :]]
                    for half in range(2):
                        k = 2 * kk + half
                        # start=True clears has_written for the WHOLE bank, so
                        # only the first matmul into each gu bank may set it;
                        # the first w3 matmul overwrites via has_written=0.
                        for m in range(MT):
                            nc.tensor.matmul(
                                gu[m][:, 0:256],
                                lhsT=halves[half][:, m * P:(m + 1) * P],
                                rhs=xk(k), start=(k == 0),
                                stop=(k == KT - 1), skip_group_check=True)
                            nc.tensor.matmul(
                                gu[m][:, 256:512],
                                lhsT=halves[half][:, HID + m * P:
                                                  HID + (m + 1) * P],
                                rhs=xk(k), start=False,
                                stop=(k == KT - 1), skip_group_check=True)

                # ---- h^T = silu(g^T) * u^T -> fp16 ----
                hT = []
                for m in range(MT):
                    gs = gs_p.tile([P, 256], F32, tag="gs")
                    nc.scalar.activation(gs[:], gu[m][:, 0:256], SILU,
                                         bias=zbias[:])
                    ht = hT_p.tile([P, 256], F16, tag="hT")
                    hT.append(ht)
                    nc.vector.tensor_tensor(ht[:], gs[:], gu[m][:, 256:512],
                                            MULT)

                # ---- stage 2: out = h @ w2, n-major, streaming out ----
                for n in range(NCH):
                    w2t = w2_p.tile([P, KT2, 512], F16, tag="w2")
                    row0 = (e * NCH + n) * KT2 * P
                    nc.sync.dma_start(
                        out=w2t[:],
                        in_=w2_d[row0:row0 + KT2 * P, :].rearrange(
                            "(k p) c -> p k c", p=P),
                    )
                    last = (e == E_PER_CORE - 1 and n == NCH - 1)
                    ob = osb_p.tile([P, 2, 512], F16, tag="osb")
                    for m2 in range(2):
                        ops = ps_p.tile([P, 512], F32, tag="ps")
                        for k2 in range(KT2):
                            nc.tensor.matmul(
                                ops[:],
                                lhsT=hT[k2][:, m2 * P:(m2 + 1) * P],
                                rhs=w2t[:, k2, :],
                                start=(k2 == 0), stop=(k2 == KT2 - 1))
                        nc.vector.tensor_copy(ob[:, m2, :], ops[:])
                        if last:
                            # split the final store so the kernel's last DMA
                            # is small (and on scalar: HWDGE, no SWDGE fixed
                            # cost): shortens the drain tail
                            nc.scalar.dma_start(
                                out=out_d[tok0 + m2 * P:tok0 + (m2 + 1) * P,
                                          n * 512:(n + 1) * 512],
                                in_=ob[:, m2, :],
                            )
                    if not last:
                        nc.gpsimd.dma_start(
                            out=out_d[tok0:tok0 + TOK_PER_E,
                                      n * 512:(n + 1) * 512].rearrange(
                                "(m p) c -> p m c", p=P),
                            in_=ob[:],
                        )
                    if n == 1 and e + 1 < E_PER_CORE:
                        # prefetch next expert's xT now: mutex arrival after
                        # this expert's w2 loads, well before stage 1 of e+1
                        xts[e + 1] = load_xT_whole(e + 1)

    nc.compile()
    return nc


def _get_program():
    if "nc" not in _program_cache:
        _program_cache["nc"] = _build_program()
    return _program_cache["nc"]


def _make_in_maps(x, w1, w2, w3):
    """Host repack: shard + transpose x, pack fp16 weights per core."""
    E = w1.shape[0]
    # xT[e, k, p, t] = x[e*256 + t, k*128 + p], fp16
    xT = np.ascontiguousarray(
        x.reshape(E, TOK_PER_E, KT, P).transpose(0, 2, 3, 1)
    ).astype(np.float16)
    # w13[e, k, p, :] = [w1[e, kP+p, :] | w3[e, kP+p, :]]
    w13 = np.concatenate(
        [w1.reshape(E, KT, P, HID), w3.reshape(E, KT, P, HID)],
        axis=3).astype(np.float16)
    # w2p[e, n, k2, p, :] = w2[e, k2*P + p, n*512:(n+1)*512]
    w2p = np.ascontiguousarray(
        w2.reshape(E, KT2, P, NCH, 512).transpose(0, 3, 1, 2, 4)
    ).astype(np.float16)

    in_maps = []
    for c in range(N_CORES):
        e0 = c * E_PER_CORE
        in_maps.append({
            "xT": xT[e0:e0 + E_PER_CORE].reshape(E_PER_CORE * KT * P,
                                                 TOK_PER_E),
            "w13": w13[e0:e0 + E_PER_CORE].reshape(E_PER_CORE * KT * P,
                                                   2 * HID),
            "w2p": w2p[e0:e0 + E_PER_CORE].reshape(E_PER_CORE * NCH * KT2 * P,
                                                   512),
        })
    return in_maps


def kernel(x, w1, w2, w3, num_local_tokens_per_expert=None, **_unused):
    x = np.asarray(x, dtype=np.float32)
    w1 = np.asarray(w1, dtype=np.float32)
    w2 = np.asarray(w2, dtype=np.float32)
    w3 = np.asarray(w3, dtype=np.float32)

    E = w1.shape[0]
    assert E == N_CORES * E_PER_CORE and x.shape == (E * TOK_PER_E, DIM)

    in_maps = _make_in_maps(x, w1, w2, w3)
    nc = _get_program()
    res = run_bass_kernel_spmd(nc, in_maps, list(range(N_CORES)))
    return np.concatenate(
        [res.results[c]["out"].astype(np.float32) for c in range(N_CORES)],
        axis=0)


# revision 19
# speedup vs baseline: 30.0598x; 30.0598x over previous
"""Grouped-experts SwiGLU MoE kernel for 8 Trainium2 NeuronCores.

Problem: x[16384, 2048] routed to 64 experts (256 contiguous tokens each);
per expert e: out_e = (silu(x_e @ w1[e]) * (x_e @ w3[e])) @ w2[e].

Sharding: expert-parallel. Core c owns experts 8c..8c+7 and therefore tokens
[2048c, 2048(c+1)) — tokens are pre-permuted so no all-to-all is needed: each
core computes its own token slice fully locally.

This kernel is balanced at the DMA/PE intersection: fp16 weights/activations
halve HBM traffic vs fp32 (112 MiB/core -> ~326 us at 360 GB/s) while fp16
matmuls run at the full 1 row/cycle PE rate (98304 rows/expert ->
~328 us/core at 2.4 GHz). x is pre-transposed on the host so no PE
transposes are needed on device.

Per-core device program (8 experts), engine/queue assignment: PE matmuls;
sync(SP) queue carries only weight DMAs; scalar(ACT) silu + final stores;
gpsimd(SWDGE) xT loads + streaming out stores; DVE gate-mul + PSUM copies.
  startup: ~30 dummy matmuls warm the PE clock gate (HAM 1.2->2.4 GHz)
  during the initial weight-DMA fill; expert 0's xT is split into 8
  independent small tiles so the first real matmul waits only for one
  small DMA + the first w13 chunk.
  per expert:
    1. xT[e] (host-pretransposed [dim, tok] k-tiles, 1 MiB fp16) was
       prefetched during the previous expert's stage 2.
    2. gu accumulation (w13 stationary, xT moving, fp16 matmuls):
       g^T/u^T [128 hid, 256 tok] accumulate over 16 k-tiles in 8 PSUM
       banks; w13 streams in 1 MiB 2-k-tile chunks, 6 bufs deep so a PE
       hiccup never stalls the DMA pipe.
    3. h^T = silu(g^T) * u^T  (ACT silu + DVE mul) -> fp16 SBUF
    4. out = h @ w2 via (hT stationary, w2 moving): out[tok, dim] natural
       layout, accumulated over 8 hidden k-tiles, n-major; each 512-wide
       n-chunk is DMA'd out as soon as it is copied; the very last block
       runs as two 256-col groups to shorten the drain tail.
Weights are host-repacked fp16 so every DMA moves contiguous >=512B chunks.
Accuracy: fp16 quantization of w/x/out gives ~5e-4 rel err.
Cost-model exec time: 334.9 us/core (PE busy 98.9%; fp32 baseline 623 us).
"""

import numpy as np

import concourse.bacc as bacc
import concourse.mybir as mybir
from concourse.bass_utils import run_bass_kernel_spmd

N_CORES = 8
E_PER_CORE = 8          # experts per core
TOK_PER_E = 256         # tokens per expert
DIM = 2048
HID = 1024
P = 128
KT = DIM // P           # 16 k-tiles (contraction over dim)
KT2 = HID // P          # 8 k-tiles (contraction over hidden)
MT = HID // P           # 8 hidden m-tiles in stage 1
NCH = DIM // 512        # 4 output n-chunks of 512 in stage 2

F32 = mybir.dt.float32
F16 = mybir.dt.float16
SILU = mybir.ActivationFunctionType.Silu
MULT = mybir.AluOpType.mult

_program_cache = {}


def _build_program():
    """Per-core Bass program. Same program for all 8 cores (SPMD)."""
    from concourse.tile import TileContext

    nc = bacc.Bacc("TRN2", target_bir_lowering=False, debug=False)

    # xT: per (e, k-tile): [128 dim, 256 tok] fp16, host-pretransposed
    xT_d = nc.dram_tensor("xT", [E_PER_CORE * KT * P, TOK_PER_E], F16,
                          kind="ExternalInput")
    # w13: per (e, k-tile): [128, 2048] = [w1 k-block | w3 k-block], contiguous
    w13_d = nc.dram_tensor("w13", [E_PER_CORE * KT * P, 2 * HID], F16,
                           kind="ExternalInput")
    # w2p: per (e, n, k2): [128, 512] contiguous; (e, n) group = 1 MiB fp16
    w2_d = nc.dram_tensor("w2p", [E_PER_CORE * NCH * KT2 * P, 512], F16,
                          kind="ExternalInput")
    out_d = nc.dram_tensor("out", [E_PER_CORE * TOK_PER_E, DIM], F16,
                           kind="ExternalOutput")

    with TileContext(nc) as tc:
        with tc.tile_pool(name="const", bufs=1) as constp, \
             tc.tile_pool(name="xT", bufs=2) as xT_p, \
             tc.tile_pool(name="xT0", bufs=8) as xT0_p, \
             tc.tile_pool(name="w13", bufs=6) as w13_p, \
             tc.tile_pool(name="w2", bufs=4) as w2_p, \
             tc.tile_pool(name="hT", bufs=16) as hT_p, \
             tc.tile_pool(name="gs", bufs=3) as gs_p, \
             tc.tile_pool(name="osb", bufs=3) as osb_p, \
             tc.tile_pool(name="ps", bufs=8, space="PSUM") as ps_p:

            zbias = constp.tile([P, 1], F32)
            nc.vector.memset(zbias, 0.0)
            warm = constp.tile([P, P], F16)
            nc.vector.memset(warm, 0.0)

            # Warm the PE clock gate (HAM) with dummy matmuls during the
            # initial DMA fill: ~3.5us of back-to-back PE activity lifts the
            # clock from 1.2GHz to 2.4GHz before the real matmuls start.
            wps = ps_p.tile([P, 512], F32, tag="ps")
            for _ in range(30):
                nc.tensor.matmul(wps[:, 0:P], lhsT=warm[:], rhs=warm[:],
                                 start=True, stop=True)

            # xT[e] in SBUF: list of (tile, k0, klen) chunks, loaded on the
            # DVE queue (sync carries only weights; scalar only silu).
            # Expert 0 uses 8 independent 2-k-tile chunks so the first
            # matmul only waits for one small DMA; later experts load one
            # tile in 4 chunk DMAs, prefetched during the previous expert's
            # stage 2 (after its w2 loads are underway).
            def load_xT_whole(e):
                xt = xT_p.tile([P, KT, TOK_PER_E], F16, tag="xT")
                for c in range(4):
                    k0 = 4 * c
                    r0 = (e * KT + k0) * P
                    nc.gpsimd.dma_start(
                        out=xt[:, k0:k0 + 4, :],
                        in_=xT_d[r0:r0 + 4 * P, :].rearrange(
                            "(k p) t -> p k t", p=P),
                    )
                return [(xt, 0, KT)]

            def load_xT_chunked(e):
                chunks = []
                for c in range(KT // 2):
                    xt = xT0_p.tile([P, 2, TOK_PER_E], F16, tag="xT0")
                    r0 = (e * KT + 2 * c) * P
                    nc.gpsimd.dma_start(
                        out=xt[:],
                        in_=xT_d[r0:r0 + 2 * P, :].rearrange(
                            "(k p) t -> p k t", p=P),
                    )
                    chunks.append((xt, 2 * c, 2))
                return chunks

            xts = [None] * E_PER_CORE

            for e in range(E_PER_CORE):
                tok0 = e * TOK_PER_E
                xch = xts[e] if xts[e] is not None else load_xT_chunked(e)

                def xk(k):
                    for tile, k0, klen in xch:
                        if k0 <= k < k0 + klen:
                            return tile[:, k - k0, :]
                    raise AssertionError

                # ---- stage 1: g/u accumulation over dim ----
                gu = [ps_p.tile([P, 512], F32, tag="ps", name=f"gu_e{e}_m{m}")
                      for m in range(MT)]
                for kk in range(KT // 2):
                    wt = w13_p.tile([P, 2, 2 * HID], F16, tag="w13")
                    row0 = (e * KT + 2 * kk) * P
                    nc.sync.dma_start(
                        out=wt[:],
                        in_=w13_d[row0:row0 + 2 * P, :].rearrange(
                            "(h p) c -> p h c", p=P),
                    )
                    halves = [wt[:, 0, :], wt[:, 1, :]]
                    for half in range(2):
                        k = 2 * kk + half
                        # start=True clears has_written for the WHOLE bank, so
                        # only the first matmul into each gu bank may set it;
                        # the first w3 matmul overwrites via has_written=0.
                        for m in range(MT):
                            nc.tensor.matmul(
                                gu[m][:, 0:256],
                                lhsT=halves[half][:, m * P:(m + 1) * P],
                                rhs=xk(k), start=(k == 0),
                                stop=(k == KT - 1), skip_group_check=True)
                            nc.tensor.matmul(
                                gu[m][:, 256:512],
                                lhsT=halves[half][:, HID + m * P:
                                                  HID + (m + 1) * P],
                                rhs=xk(k), start=False,
                                stop=(k == KT - 1), skip_group_check=True)

                # ---- h^T = silu(g^T) * u^T -> fp16 ----
                hT = []
                for m in range(MT):
                    gs = gs_p.tile([P, 256], F32, tag="gs")
                    nc.scalar.activation(gs[:], gu[m][:, 0:256], SILU,
                                         bias=zbias[:])
                    ht = hT_p.tile([P, 256], F16, tag="hT")
                    hT.append(ht)
                    nc.vector.tensor_tensor(ht[:], gs[:], gu[m][:, 256:512],
                                            MULT)

                # ---- stage 2: out = h @ w2, n-major, streaming out ----
                for n in range(NCH):
                    w2t = w2_p.tile([P, KT2, 512], F16, tag="w2")
                    row0 = (e * NCH + n) * KT2 * P
                    nc.sync.dma_start(
                        out=w2t[:],
                        in_=w2_d[row0:row0 + KT2 * P, :].rearrange(
                            "(k p) c -> p k c", p=P),
                    )
                    last = (e == E_PER_CORE - 1 and n == NCH - 1)
                    ob = osb_p.tile([P, 2, 512], F16, tag="osb")
                    for m2 in range(2):
                        if last and m2 == 1:
                            # final block: two 256-col accumulation groups so
                            # the last PSUM->SBUF copy is half-size and
                            # overlaps the second group's matmuls
                            for ch in range(2):
                                ops = ps_p.tile([P, 512], F32, tag="ps")
                                c0 = ch * 256
                                for k2 in range(KT2):
                                    nc.tensor.matmul(
                                        ops[:, 0:256],
                                        lhsT=hT[k2][:, m2 * P:(m2 + 1) * P],
                                        rhs=w2t[:, k2, c0:c0 + 256],
                                        start=(k2 == 0), stop=(k2 == KT2 - 1))
                                nc.vector.tensor_copy(
                                    ob[:, m2, c0:c0 + 256], ops[:, 0:256])
                                nc.scalar.dma_start(
                                    out=out_d[tok0 + m2 * P:
                                              tok0 + (m2 + 1) * P,
                                              n * 512 + c0:n * 512 + c0 + 256],
                                    in_=ob[:, m2, c0:c0 + 256],
                                )
                            continue
                        ops = ps_p.tile([P, 512], F32, tag="ps")
                        for k2 in range(KT2):
                            nc.tensor.matmul(
                                ops[:],
                                lhsT=hT[k2][:, m2 * P:(m2 + 1) * P],
                                rhs=w2t[:, k2, :],
                                start=(k2 == 0), stop=(k2 == KT2 - 1))
                        nc.vector.tensor_copy(ob[:, m2, :], ops[:])
                        if last:
                            # split the final store so the kernel's last DMA
                            # is small (and on scalar: HWDGE, no SWDGE fixed
                            # cost): shortens the drain tail
                            nc.scalar.dma_start(
                                out=out_d[tok0 + m2 * P:tok0 + (m2 + 1) * P,
                                          n * 512:(n + 1) * 512],
                                in_=ob[:, m2, :],
                            )
                    if not last:
                        nc.gpsimd.dma_start(
                            out=out_d[tok0:tok0 + TOK_PER_E,
                                      n * 512:(n + 1) * 512].rearrange(
                                "(m p) c -> p m c", p=P),
                            in_=ob[:],
                        )
                    if n == 1 and e + 1 < E_PER_CORE:
                        # prefetch next expert's xT now: mutex arrival after
                        # this expert's w2 loads, well before stage 1 of e+1
                        xts[e + 1] = load_xT_whole(e + 1)

    nc.compile()
    return nc


def _get_program():
    if "nc" not in _program_cache:
        _program_cache["nc"] = _build_program()
    return _program_cache["nc"]


def _make_in_maps(x, w1, w2, w3):
    """Host repack: shard + transpose x, pack fp16 weights per core."""
    E = w1.shape[0]
    # xT[e, k, p, t] = x[e*256 + t, k*128 + p], fp16
    xT = np.ascontiguousarray(
        x.reshape(E, TOK_PER_E, KT, P).transpose(0, 2, 3, 1)
    ).astype(np.float16)
    # w13[e, k, p, :] = [w1[e, kP+p, :] | w3[e, kP+p, :]]
    w13 = np.concatenate(
        [w1.reshape(E, KT, P, HID), w3.reshape(E, KT, P, HID)],
        axis=3).astype(np.float16)
    # w2p[e, n, k2, p, :] = w2[e, k2*P + p, n*512:(n+1)*512]
    w2p = np.ascontiguousarray(
        w2.reshape(E, KT2, P, NCH, 512).transpose(0, 3, 1, 2, 4)
    ).astype(np.float16)

    in_maps = []
    for c in range(N_CORES):
        e0 = c * E_PER_CORE
        in_maps.append({
            "xT": xT[e0:e0 + E_PER_CORE].reshape(E_PER_CORE * KT * P,
                                                 TOK_PER_E),
            "w13": w13[e0:e0 + E_PER_CORE].reshape(E_PER_CORE * KT * P,
                                                   2 * HID),
            "w2p": w2p[e0:e0 + E_PER_CORE].reshape(E_PER_CORE * NCH * KT2 * P,
                                                   512),
        })
    return in_maps


def kernel(x, w1, w2, w3, num_local_tokens_per_expert=None, **_unused):
    x = np.asarray(x, dtype=np.float32)
    w1 = np.asarray(w1, dtype=np.float32)
    w2 = np.asarray(w2, dtype=np.float32)
    w3 = np.asarray(w3, dtype=np.float32)

    E = w1.shape[0]
    assert E == N_CORES * E_PER_CORE and x.shape == (E * TOK_PER_E, DIM)

    in_maps = _make_in_maps(x, w1, w2, w3)
    nc = _get_program()
    res = run_bass_kernel_spmd(nc, in_maps, list(range(N_CORES)))
    return np.concatenate(
        [res.results[c]["out"].astype(np.float32) for c in range(N_CORES)],
        axis=0)
